# revision 1
# baseline (speedup 1.0000x reference)
import os, sys, math
import numpy as np

sys.path.insert(0, "/opt/trn_rl_repo")

import concourse.bass as bass
import concourse.bacc as bacc
import concourse.mybir as mybir
from concourse import tile
from concourse import bass_utils

F32 = mybir.dt.float32
BF16 = mybir.dt.bfloat16
ALU = mybir.AluOpType
ACTF = mybir.ActivationFunctionType
AX = mybir.AxisListType

N_ATOMS = 50000
N_CORES = 8
N_PAD = 51200            # 8 * 6400
APC = 6400               # atoms per core
WPC = 50                 # 128-atom windows per core
GRP = 512
R_MAX = 6.0
LN_HALF = math.log(0.5)
S330 = 1.0 / math.sqrt(330.0)
S64 = 1.0 / math.sqrt(64.0)
S512 = 1.0 / math.sqrt(512.0)


def _pack_edges(key_idx, dr, C):
    """Sort edges by key atom, bucket into 128-atom windows, pad each window to
    C chunks of 128. rel pad = 0 when dr given (features are 0), else -1."""
    order = np.argsort(key_idx, kind="stable")
    k_s = key_idx[order]
    win = (k_s >> 7).astype(np.int64)
    cnt = np.bincount(win, minlength=400)
    start = np.zeros(400, np.int64)
    start[1:] = np.cumsum(cnt)[:-1]
    rank = np.arange(len(k_s)) - start[win]
    p = rank % 128
    c = rank // 128
    pad = 0.0 if dr is not None else -1.0
    relP = np.full((400, 128, C), pad, np.float32)
    relP[win, p, c] = (k_s & 127).astype(np.float32)
    drP = None
    if dr is not None:
        drP = np.zeros((400, 128, C, 3), np.float32)
        drP[..., 0] = 100.0
        drP[win, p, c] = dr[order]
    return drP, relP


ENG_GUU = "gpsimd"   # guu+guuu outer products
ENG_C3 = "vector"    # c3 expansion mult
ENG_TF = "gpsimd"    # F expansion mult


def _build_program(C, Cj):
    nc = bacc.Bacc("TRN2", target_bir_lowering=False, debug=False)

    for v in (math.pi / 2, LN_HALF):
        t = nc.alloc_sbuf_tensor(f"constx{len(nc.const_aps.aps)}", [128, 1], F32)
        nc.gpsimd.memset(t.ap(), v)
        nc.const_aps.aps[(F32, v)] = t.ap()
    nc.all_engine_barrier()

    dram = {}
    for name, shape in [
        ("dr", [WPC, 128, C * 3]), ("irel", [WPC, 128, C]),
        ("jrel", [WPC, 128, Cj]), ("zf", [128, WPC]), ("scsh", [128, 240]),
        ("iota", [128, 128]), ("ident", [128, 128]), ("cent", [128, 5]),
        ("ones", [128, 64]), ("wn", [330, 64]), ("wm", [64, 64]),
        ("w1", [64, 512]), ("w2", [512, 512]), ("w3", [512, 1]),
        ("b1", [512, 1]), ("b2", [512, 1]), ("b3", [1, 1]),
    ]:
        dram[name] = nc.dram_tensor(name, shape, F32, kind="ExternalInput").ap()
    d_out = nc.dram_tensor("out", [APC], F32, kind="ExternalOutput").ap()
    d_stage = nc.dram_tensor("stage", [1, APC], F32, kind="Internal").ap()
    d_deg = nc.dram_tensor("degst", [1, APC], F32, kind="Internal").ap()

    with tile.TileContext(nc) as tc:
        from contextlib import ExitStack
        with ExitStack() as ctx:
            P = lambda n, b, **kw: ctx.enter_context(tc.tile_pool(name=n, bufs=b, **kw))
            cpool = P("consts", 1)
            allp = P("allwin", 1)      # whole-run big tiles
            epool = P("edges", 2)
            fpool = P("feat", 2)
            oipool = P("ohi", 2)
            ojpool = P("ohj", 2)
            spool = P("escr", 2)
            mpool = P("mom", 2)
            gsp = P("gmscr", 2)
            tpool = P("cscr", 2)
            dpool = P("df", 2)
            hpool = P("hmsg", 2)
            apool = P("a1", 1)
            a2pool = P("a2", 1)
            finpool = P("fin", 1)
            pm = P("pm", 2, space="PSUM")
            pd = P("pd", 1, space="PSUM")
            pt = P("pt", 2, space="PSUM")
            pmlp = P("pmlp", 2, space="PSUM")
            ph = P("ph", 1, space="PSUM")

            _ld = [0]

            def load(pool, shape, src, dt=F32):
                _ld[0] += 1
                t = pool.tile(list(shape), dt, name=f"ld{_ld[0]}",
                              tag=f"ld{_ld[0]}")
                nc.sync.dma_start(t[:, :], src)
                return t

            iota = load(cpool, [128, 128], dram["iota"][:, :])
            ident = load(cpool, [128, 128], dram["ident"][:, :])
            cent = load(cpool, [128, 5], dram["cent"][:, :])
            ones = load(cpool, [128, 64], dram["ones"][:, :])
            scsh = load(cpool, [128, 240], dram["scsh"][:, :])
            zf = load(cpool, [128, WPC], dram["zf"][:, :])
            wm = load(cpool, [64, 64], dram["wm"][:, :])
            w1 = load(cpool, [64, 512], dram["w1"][:, :])
            w2t = [load(cpool, [128, 512], dram["w2"][k * 128:(k + 1) * 128, :])
                   for k in range(4)]
            w3t = [load(cpool, [128, 1], dram["w3"][k * 128:(k + 1) * 128, :])
                   for k in range(4)]
            b1 = load(cpool, [128, 4], dram["b1"].rearrange("(m p) q -> p (m q)", p=128))
            b2 = load(cpool, [128, 4], dram["b2"].rearrange("(m p) q -> p (m q)", p=128))
            b3 = load(cpool, [1, 1], dram["b3"][:, :])
            # bf16 casts of constants used in bf16 matmuls
            iotaB = cpool.tile([128, 128], BF16)
            nc.vector.tensor_copy(iotaB[:, :], iota[:, :])
            identB = cpool.tile([128, 128], BF16)
            nc.vector.tensor_copy(identB[:, :], ident[:, :])
            wnf = [load(cpool, [128, 64], dram["wn"][0:128, :]),
                   load(cpool, [128, 64], dram["wn"][128:256, :]),
                   load(cpool, [74, 64], dram["wn"][256:330, :])]
            wnB = []
            for k, t in enumerate(wnf):
                p = t.shape[0]
                tb = cpool.tile([p, 64], BF16, name=f"wnB{k}", tag=f"wnB{k}")
                nc.vector.tensor_copy(tb[:, :], t[:, :])
                wnB.append(tb)

            # whole-run tiles
            irAll = allp.tile([128, WPC * C], F32)
            jrAll = allp.tile([128, WPC * Cj], F32)
            for w in range(WPC):
                nc.sync.dma_start(irAll[:, w * C:(w + 1) * C], dram["irel"][w])
                nc.sync.dma_start(jrAll[:, w * Cj:(w + 1) * Cj], dram["jrel"][w])
            rAll = allp.tile([128, WPC * C], F32)
            cutAll = allp.tile([128, WPC * C], F32)
            uAll = allp.tile([128, WPC * C * 3], BF16)
            gmAll = allp.tile([128, WPC * 330], BF16)

            # ---------------- loop A: radii + unit vectors ----------------
            for w in range(WPC):
                dr = epool.tile([128, C * 3], F32, tag="dr")
                nc.sync.dma_start(dr[:, :], dram["dr"][w])
                dr3 = dr[:, :].rearrange("p (c x) -> p c x", x=3)
                sq = spool.tile([128, C * 3], F32, tag="sq")
                nc.vector.tensor_tensor(sq[:, :], dr[:, :], dr[:, :], ALU.mult)
                r = rAll[:, w * C:(w + 1) * C]
                nc.vector.tensor_reduce(
                    r, sq[:, :].rearrange("p (c x) -> p c x", x=3),
                    axis=AX.X, op=ALU.add)
                nc.scalar.activation(r, r, ACTF.Sqrt)
                rp = spool.tile([128, C], F32, tag="rp")
                nc.vector.tensor_scalar_add(rp[:, :], r, 1e-9)
                rinv = spool.tile([128, C], F32, tag="rinv")
                nc.vector.reciprocal(rinv[:, :], rp[:, :])
                u3 = uAll[:, w * C * 3:(w + 1) * C * 3].rearrange(
                    "p (c x) -> p c x", x=3)
                nc.vector.tensor_tensor(
                    u3, dr3, rinv[:, :].unsqueeze(2).broadcast_to((128, C, 3)),
                    ALU.mult)
                nc.vector.tensor_scalar_min(r, r, 6.75)

            # ---------------- loop B: cutoff (Sin table) ----------------
            for w in range(WPC):
                r = rAll[:, w * C:(w + 1) * C]
                sin = spool.tile([128, C], F32, tag="sin")
                nc.scalar.activation(sin[:, :], r, ACTF.Sin,
                                     bias=math.pi / 2, scale=-math.pi / R_MAX)
                m01 = spool.tile([128, C], F32, tag="m01")
                nc.vector.tensor_single_scalar(m01[:, :], r, R_MAX, ALU.is_lt)
                nc.vector.scalar_tensor_tensor(
                    cutAll[:, w * C:(w + 1) * C], sin[:, :], 1.0, m01[:, :],
                    ALU.add, ALU.mult)

            # ------------- loop C: features, scatter, contraction -------------
            for w in range(WPC):
                r = rAll[:, w * C:(w + 1) * C]
                u3 = uAll[:, w * C * 3:(w + 1) * C * 3].rearrange(
                    "p (c x) -> p c x", x=3)
                cut2 = cutAll[:, w * C:(w + 1) * C]
                d5 = spool.tile([128, C * 5], F32, tag="d5")
                d5v = d5[:, :].rearrange("p (c k) -> p c k", k=5)
                nc.vector.tensor_tensor(
                    d5v, r.unsqueeze(2).broadcast_to((128, C, 5)),
                    cent[:, :].unsqueeze(1).broadcast_to((128, C, 5)),
                    ALU.subtract)
                nc.scalar.activation(d5[:, :], d5[:, :], ACTF.Square)
                nc.scalar.activation(d5[:, :], d5[:, :], ACTF.Exp,
                                     bias=LN_HALF, scale=-1.0)
                nc.vector.tensor_tensor(
                    d5v, d5v, cut2.unsqueeze(2).broadcast_to((128, C, 5)),
                    ALU.mult)
                F = fpool.tile([128, C * 200], BF16, tag="F")
                Fv = F[:, :].rearrange("p (c f) -> p c f", f=200)
                nc.vector.tensor_copy(Fv[:, :, 0:5], d5v)
                nc.vector.tensor_tensor(
                    Fv[:, :, 5:20].rearrange("p c (r x) -> p c r x", x=3),
                    Fv[:, :, 0:5].unsqueeze(3).broadcast_to((128, C, 5, 3)),
                    u3.unsqueeze(2).broadcast_to((128, C, 5, 3)), ALU.mult)
                getattr(nc, ENG_GUU).tensor_tensor(
                    Fv[:, :, 20:65].rearrange("p c (r x) -> p c r x", x=3),
                    Fv[:, :, 5:20].unsqueeze(3).broadcast_to((128, C, 15, 3)),
                    u3.unsqueeze(2).broadcast_to((128, C, 15, 3)), ALU.mult)
                getattr(nc, ENG_GUU).tensor_tensor(
                    Fv[:, :, 65:200].rearrange("p c (r x) -> p c r x", x=3),
                    Fv[:, :, 20:65].unsqueeze(3).broadcast_to((128, C, 45, 3)),
                    u3.unsqueeze(2).broadcast_to((128, C, 45, 3)), ALU.mult)

                ohi = oipool.tile([128, C * 128], BF16, tag="ohi")
                for c in range(C):
                    nc.vector.tensor_scalar(
                        ohi[:, c * 128:(c + 1) * 128], iotaB[:, :],
                        irAll[:, w * C + c:w * C + c + 1], None, ALU.is_equal)
                ohj = ojpool.tile([128, Cj * 128], F32, tag="ohj")
                for c in range(Cj):
                    nc.gpsimd.tensor_scalar(
                        ohj[:, c * 128:(c + 1) * 128], iota[:, :],
                        jrAll[:, w * Cj + c:w * Cj + c + 1], None, ALU.is_equal)

                mom_ps = pm.tile([128, 200], F32)
                for c in range(C):
                    nc.tensor.matmul(mom_ps[:, :], ohi[:, c * 128:(c + 1) * 128],
                                     F[:, c * 200:(c + 1) * 200],
                                     start=(c == 0), stop=(c == C - 1))
                deg_ps = pd.tile([1, 128], F32)
                for c in range(Cj):
                    nc.tensor.matmul(deg_ps[:, :], ones[:, 0:1],
                                     ohj[:, c * 128:(c + 1) * 128],
                                     start=(c == 0), stop=(c == Cj - 1))
                mom = mpool.tile([128, 200], F32)
                nc.scalar.activation(mom[:, :], mom_ps[:, :], ACTF.Copy)
                degsb = spool.tile([1, 128], F32, tag="degsb")
                nc.scalar.activation(degsb[:, :], deg_ps[:, :], ACTF.Copy)
                nc.sync.dma_start(d_deg[0:1, w * 128:(w + 1) * 128], degsb[:, :])

                # ---- contraction -> gm ----
                gm = gsp.tile([128, 330], F32, tag="gm")
                momv = mom[:, :]
                m1 = momv[:, 5:20].rearrange("p (r i) -> p r i", r=5)
                m2 = momv[:, 20:65].rearrange("p (r a) -> p r a", r=5)
                m2ji = momv[:, 20:65].rearrange("p (r i j) -> p r j i", r=5, i=3)
                m3 = momv[:, 65:200].rearrange("p (r a) -> p r a", r=5)
                m3k = momv[:, 65:200].rearrange("p (r ij k) -> p r k ij", r=5, ij=9)
                nc.vector.tensor_copy(gm[:, 0:5], momv[:, 0:5])
                t1 = tpool.tile([128, 675], F32, tag="big")
                nc.vector.tensor_tensor(
                    t1[:, 0:75].rearrange("p (r s i) -> p r s i", r=5, s=5),
                    m1.unsqueeze(2).broadcast_to((128, 5, 5, 3)),
                    m1.unsqueeze(1).broadcast_to((128, 5, 5, 3)), ALU.mult)
                nc.vector.tensor_reduce(
                    gm[:, 5:30], t1[:, 0:75].rearrange("p (q i) -> p q i", i=3),
                    axis=AX.X, op=ALU.add)
                t2 = tpool.tile([128, 675], F32, tag="big")
                nc.vector.tensor_tensor(
                    t2[:, 0:225].rearrange("p (r s a) -> p r s a", r=5, s=5),
                    m2.unsqueeze(2).broadcast_to((128, 5, 5, 9)),
                    m2.unsqueeze(1).broadcast_to((128, 5, 5, 9)), ALU.mult)
                nc.vector.tensor_reduce(
                    gm[:, 30:55], t2[:, 0:225].rearrange("p (q a) -> p q a", a=9),
                    axis=AX.X, op=ALU.add)
                t3 = tpool.tile([128, 675], F32, tag="big")
                getattr(nc, ENG_C3).tensor_tensor(
                    t3[:, :].rearrange("p (r s a) -> p r s a", r=5, s=5),
                    m3.unsqueeze(2).broadcast_to((128, 5, 5, 27)),
                    m3.unsqueeze(1).broadcast_to((128, 5, 5, 27)), ALU.mult)
                nc.vector.tensor_reduce(
                    gm[:, 55:80], t3[:, :].rearrange("p (q a) -> p q a", a=27),
                    axis=AX.X, op=ALU.add)
                tD = tpool.tile([128, 675], F32, tag="big")
                for rr in range(5):
                    nc.vector.tensor_tensor(
                        tD[:, rr * 45:(rr + 1) * 45].rearrange(
                            "p (s j i) -> p s j i", s=5, j=3),
                        m2ji[:, rr].unsqueeze(1).broadcast_to((128, 5, 3, 3)),
                        m1.unsqueeze(2).broadcast_to((128, 5, 3, 3)),
                        ALU.mult)
                D = dpool.tile([128, 75], F32, tag="D")
                nc.vector.tensor_reduce(
                    D[:, :], tD[:, 0:225].rearrange("p (q i) -> p q i", i=3),
                    axis=AX.X, op=ALU.add)
                t4 = tpool.tile([128, 675], F32, tag="big")
                for rr in range(5):
                    nc.vector.tensor_tensor(
                        t4[:, rr * 75:(rr + 1) * 75].rearrange(
                            "p (s t j) -> p s t j", s=5, t=5),
                        D[:, rr * 15:(rr + 1) * 15].rearrange(
                            "p (s j) -> p s j", s=5)
                            .unsqueeze(2).broadcast_to((128, 5, 5, 3)),
                        m1.unsqueeze(1).broadcast_to((128, 5, 5, 3)),
                        ALU.mult)
                nc.vector.tensor_reduce(
                    gm[:, 80:205], t4[:, 0:375].rearrange("p (q j) -> p q j", j=3),
                    axis=AX.X, op=ALU.add)
                tF = tpool.tile([128, 675], F32, tag="big")
                for rr in range(5):
                    getattr(nc, ENG_TF).tensor_tensor(
                        tF[:, rr * 135:(rr + 1) * 135].rearrange(
                            "p (s k ij) -> p s k ij", s=5, k=3),
                        m3k[:, rr].unsqueeze(1).broadcast_to((128, 5, 3, 9)),
                        m2.unsqueeze(2).broadcast_to((128, 5, 3, 9)),
                        ALU.mult)
                Ft = dpool.tile([128, 75], F32, tag="Ft")
                nc.vector.tensor_reduce(
                    Ft[:, :], tF[:, :].rearrange("p (q ij) -> p q ij", ij=9),
                    axis=AX.X, op=ALU.add)
                t5 = tpool.tile([128, 675], F32, tag="big")
                for rr in range(5):
                    nc.vector.tensor_tensor(
                        t5[:, rr * 75:(rr + 1) * 75].rearrange(
                            "p (s t k) -> p s t k", s=5, t=5),
                        Ft[:, rr * 15:(rr + 1) * 15].rearrange(
                            "p (s k) -> p s k", s=5)
                            .unsqueeze(2).broadcast_to((128, 5, 5, 3)),
                        m1.unsqueeze(1).broadcast_to((128, 5, 5, 3)),
                        ALU.mult)
                nc.vector.tensor_reduce(
                    gm[:, 205:330], t5[:, 0:375].rearrange("p (q k) -> p q k", k=3),
                    axis=AX.X, op=ALU.add)
                nc.vector.tensor_copy(gmAll[:, w * 330:(w + 1) * 330], gm[:, :])

            # ---------------- loop D: projection + MLP per group ----------------
            w = 0
            for g in range(13):
                nw = min(4, WPC - w)
                na = nw * 128
                gt = [hpool.tile([128, GRP], BF16, tag=f"gmt{k}", name=f"gt{k}") for k in range(3)]
                for wi in range(nw):
                    for k in range(3):
                        lo, hi = k * 128, min((k + 1) * 128, 330)
                        ps = pt.tile([128, 128], BF16)
                        nc.tensor.matmul(ps[0:hi - lo, 0:128],
                                         gmAll[:, (w + wi) * 330 + lo:(w + wi) * 330 + hi],
                                         identB[:, :], is_transpose=True,
                                         start=True, stop=True)
                        nc.scalar.activation(
                            gt[k][0:hi - lo, wi * 128:(wi + 1) * 128],
                            ps[0:hi - lo, 0:128], ACTF.Copy)
                hps = ph.tile([64, GRP], F32, tag="hps")
                nc.tensor.matmul(hps[:, 0:na], wnB[0][:, :], gt[0][:, 0:na],
                                 start=True, stop=False)
                nc.tensor.matmul(hps[:, 0:na], wnB[1][:, :], gt[1][:, 0:na],
                                 start=False, stop=False)
                nc.tensor.matmul(hps[:, 0:na], wnB[2][:, :], gt[2][0:74, 0:na],
                                 start=False, stop=True)
                hT = hpool.tile([64, GRP], F32, tag="hT")
                nc.scalar.activation(hT[:, 0:na], hps[:, 0:na], ACTF.Identity,
                                     scale=S330)
                mps = ph.tile([64, GRP], F32, tag="hps")
                nc.tensor.matmul(mps[:, 0:na], wm[:, :], hT[:, 0:na],
                                 start=True, stop=True)
                msgT = hpool.tile([64, GRP], F32, tag="msgT")
                nc.scalar.activation(msgT[:, 0:na], mps[:, 0:na], ACTF.Silu,
                                     scale=S64)
                degt = hpool.tile([1, GRP], F32, tag="degt")
                nc.sync.dma_start(degt[0:1, 0:na], d_deg[0:1, w * 128:w * 128 + na])
                dps = ph.tile([64, GRP], F32, tag="hps")
                nc.tensor.matmul(dps[:, 0:na], ones[0:1, 0:64], degt[0:1, 0:na],
                                 start=True, stop=True)
                hpT = hpool.tile([64, GRP], F32, tag="hpT")
                nc.vector.tensor_tensor(msgT[:, 0:na], msgT[:, 0:na],
                                        dps[:, 0:na], ALU.mult)
                nc.vector.tensor_tensor(hpT[:, 0:na], hT[:, 0:na],
                                        msgT[:, 0:na], ALU.add)
                a1 = [apool.tile([128, GRP], F32, tag=f"a1_{m}", name=f"a1_{m}") for m in range(4)]
                for m in range(4):
                    ps = pmlp.tile([128, GRP], F32)
                    nc.tensor.matmul(ps[:, 0:na], w1[:, m * 128:(m + 1) * 128],
                                     hpT[:, 0:na], start=True, stop=True)
                    nc.scalar.activation(a1[m][:, 0:na], ps[:, 0:na], ACTF.Silu,
                                         bias=b1[:, m:m + 1], scale=S64)
                a2 = [a2pool.tile([128, GRP], F32, tag=f"a2_{m}", name=f"a2_{m}") for m in range(4)]
                for m in range(4):
                    ps = pmlp.tile([128, GRP], F32)
                    for k in range(4):
                        nc.tensor.matmul(ps[:, 0:na],
                                         w2t[k][:, m * 128:(m + 1) * 128],
                                         a1[k][:, 0:na],
                                         start=(k == 0), stop=(k == 3))
                    nc.scalar.activation(a2[m][:, 0:na], ps[:, 0:na], ACTF.Silu,
                                         bias=b2[:, m:m + 1], scale=S512)
                ops_ = ph.tile([64, GRP], F32, tag="hps")
                for k in range(4):
                    nc.tensor.matmul(ops_[0:1, 0:na], w3t[k][:, :], a2[k][:, 0:na],
                                     start=(k == 0), stop=(k == 3))
                orow = hpool.tile([1, GRP], F32, tag="orow")
                nc.scalar.activation(orow[0:1, 0:na], ops_[0:1, 0:na],
                                     ACTF.Identity, bias=b3[0:1, 0:1], scale=S512)
                nc.sync.dma_start(d_stage[0:1, w * 128:w * 128 + na],
                                  orow[0:1, 0:na])
                w += nw

            # ---------------- epilogue: scale/shift/mask ----------------
            outM = finpool.tile([128, WPC], F32)
            nc.sync.dma_start(outM[:, :],
                              d_stage.rearrange("q (w p) -> (q p) w", p=128))
            scz = finpool.tile([128, WPC], F32)
            shz = finpool.tile([128, WPC], F32)
            msk = finpool.tile([128, WPC], F32)
            for w in range(WPC):
                zoh = finpool.tile([128, 120], F32, tag="zoh")
                nc.vector.tensor_scalar(zoh[:, :], iota[:, 0:120],
                                        zf[:, w:w + 1], None, ALU.is_equal)
                prod = finpool.tile([128, 240], F32, tag="prod")
                nc.vector.tensor_tensor(
                    prod[:, :].rearrange("p (q f) -> p q f", q=2),
                    zoh[:, :].unsqueeze(1).broadcast_to((128, 2, 120)),
                    scsh[:, :].rearrange("p (q f) -> p q f", q=2), ALU.mult)
                nc.vector.tensor_reduce(
                    scz[:, w:w + 1], prod[:, 0:120].unsqueeze(1), axis=AX.X,
                    op=ALU.add)
                nc.vector.tensor_reduce(
                    shz[:, w:w + 1], prod[:, 120:240].unsqueeze(1), axis=AX.X,
                    op=ALU.add)
            nc.vector.tensor_single_scalar(msk[:, :], zf[:, :], 0.5, ALU.is_gt)
            nc.vector.tensor_tensor(outM[:, :], outM[:, :], scz[:, :], ALU.mult)
            nc.vector.tensor_tensor(outM[:, :], outM[:, :], shz[:, :], ALU.add)
            nc.vector.tensor_tensor(outM[:, :], outM[:, :], msk[:, :], ALU.mult)
            nc.sync.dma_start(d_out.rearrange("(w p) -> p w", p=128), outM[:, :])

    nc.compile()
    return nc


_CACHED = {}


def kernel(dr_vec, Z, idx, W_node, W_msg, W_r1, b_r1, W_r2, b_r2, W_r3, b_r3,
           scale, shift):
    dr_vec = np.asarray(dr_vec, np.float32)
    Z = np.asarray(Z)
    i_idx = np.asarray(idx[0], np.int64)
    j_idx = np.asarray(idx[1], np.int64)

    C = int(np.ceil(np.bincount(i_idx >> 7, minlength=400).max() / 128))
    Cj = int(np.ceil(np.bincount(j_idx >> 7, minlength=400).max() / 128))

    drP, irelP = _pack_edges(i_idx, dr_vec, C)
    _, jrelP = _pack_edges(j_idx, None, Cj)

    Zpad = np.zeros(N_PAD, np.int64)
    Zpad[:N_ATOMS] = Z
    Zw = Zpad.reshape(400, 128)

    sc = np.zeros(120, np.float32); sc[:119] = np.asarray(scale, np.float32).ravel()
    sh = np.zeros(120, np.float32); sh[:119] = np.asarray(shift, np.float32).ravel()
    common = dict(
        scsh=np.broadcast_to(np.concatenate([sc, sh])[None, :], (128, 240)).copy(),
        iota=np.broadcast_to(np.arange(128, dtype=np.float32)[None, :],
                             (128, 128)).copy(),
        ident=np.eye(128, dtype=np.float32),
        cent=np.broadcast_to(np.linspace(0.5, R_MAX, 5, dtype=np.float32)[None, :],
                             (128, 5)).copy(),
        ones=np.ones((128, 64), np.float32),
        wn=np.asarray(W_node, np.float32), wm=np.asarray(W_msg, np.float32),
        w1=np.asarray(W_r1, np.float32), w2=np.asarray(W_r2, np.float32),
        w3=np.asarray(W_r3, np.float32),
        b1=np.asarray(b_r1, np.float32).reshape(512, 1),
        b2=np.asarray(b_r2, np.float32).reshape(512, 1),
        b3=np.asarray(b_r3, np.float32).reshape(1, 1),
    )
    in_maps = []
    for c in range(N_CORES):
        w0 = c * WPC
        m = dict(common)
        m["dr"] = np.ascontiguousarray(drP[w0:w0 + WPC].reshape(WPC, 128, C * 3))
        m["irel"] = np.ascontiguousarray(irelP[w0:w0 + WPC])
        m["jrel"] = np.ascontiguousarray(jrelP[w0:w0 + WPC])
        m["zf"] = np.ascontiguousarray(Zw[w0:w0 + WPC].T.astype(np.float32))
        in_maps.append(m)

    key = (C, Cj)
    if key not in _CACHED:
        _CACHED[key] = _build_program(C, Cj)
    nc = _CACHED[key]

    import time as _t
    t0 = _t.time()
    res = bass_utils.run_bass_kernel_spmd(
        nc, in_maps, core_ids=list(range(N_CORES)))
    t1 = _t.time()
    if os.environ.get("BENCH_TIME"):
        print(f"device run wall: {(t1 - t0) * 1e3:.1f} ms")
    if res.exec_time_ns is not None:
        print(f"HW exec time: {res.exec_time_ns} ns")
    outs = [res.results[c]["out"] for c in range(N_CORES)]
    full = np.concatenate(outs)[:N_ATOMS].astype(np.float32)
    return full[:, None]



# revision 22
# speedup vs baseline: 2.3652x; 2.3652x over previous
import os, sys, math
import numpy as np

sys.path.insert(0, "/opt/trn_rl_repo")

import concourse.bass as bass
import concourse.bacc as bacc
import concourse.mybir as mybir
from concourse import tile
from concourse import bass_utils

F32 = mybir.dt.float32
BF16 = mybir.dt.bfloat16
ALU = mybir.AluOpType
ACTF = mybir.ActivationFunctionType
AX = mybir.AxisListType

N_ATOMS = 50000
N_CORES = 8
N_PAD = 51200            # 8 * 6400
APC = 6400               # atoms per core
WPC = 50                 # 128-atom windows per core
GB = 10                  # windows per geometry batch
R_MAX = 6.0
LN_HALF = math.log(0.5)
S330 = 1.0 / math.sqrt(330.0)
S64 = 1.0 / math.sqrt(64.0)
S512 = 1.0 / math.sqrt(512.0)

# unique index orders
PAIRS = [(0, 0), (0, 1), (0, 2), (1, 1), (1, 2), (2, 2)]
TRIPS = [(0, 0, 0), (0, 0, 1), (0, 0, 2), (0, 1, 1), (0, 1, 2),
         (0, 2, 2), (1, 1, 1), (1, 1, 2), (1, 2, 2), (2, 2, 2)]
PAIR_IDX = {p: k for k, p in enumerate(PAIRS)}
TRIP_IDX = {t: k for k, t in enumerate(TRIPS)}
W6 = np.array([1, 2, 2, 1, 2, 1], np.float32)
W10 = np.array([1, 3, 3, 3, 6, 3, 1, 3, 3, 1], np.float32)


def _pack_edges(key_idx, dr, C):
    """Sort edges by center atom, bucket into 128-atom windows, pad each
    window to C chunks of 128."""
    order = np.argsort(key_idx, kind="stable")
    k_s = key_idx[order]
    win = (k_s >> 7).astype(np.int64)
    cnt = np.bincount(win, minlength=400)
    start = np.zeros(400, np.int64)
    start[1:] = np.cumsum(cnt)[:-1]
    rank = np.arange(len(k_s)) - start[win]
    p = rank % 128
    c = rank // 128
    relP = np.zeros((400, 128, C), np.float32)
    relP[win, p, c] = (k_s & 127).astype(np.float32)
    drP = np.zeros((400, 128, C, 3), np.float32)
    drP[..., 0] = 100.0
    drP[win, p, c] = dr[order]
    return drP, relP


def _build_E():
    """[80, 260] expansion matrix: momuT rows (r,p6 | r,t10) ->
    [m2f 45 (r,j,i) | m3kij 135 (r,k,ij) | m2uw 30 | m3uw 50]."""
    E = np.zeros((80, 260), np.float32)
    for r in range(5):
        for j in range(3):
            for i in range(3):
                E[6 * r + PAIR_IDX[tuple(sorted((j, i)))], 9 * r + 3 * j + i] = 1.0
        for k in range(3):
            for i in range(3):
                for j in range(3):
                    t = TRIP_IDX[tuple(sorted((i, j, k)))]
                    E[30 + 10 * r + t, 45 + 27 * r + 9 * k + 3 * i + j] = 1.0
        for p in range(6):
            E[6 * r + p, 180 + 6 * r + p] = W6[p]
        for t in range(10):
            E[30 + 10 * r + t, 210 + 10 * r + t] = W10[t]
    return E


def _build_program(C):
    C3 = 3 * C
    C4 = 4 * C
    NB = WPC // GB
    nc = bacc.Bacc("TRN2", target_bir_lowering=False, debug=False)

    for v in (math.pi / 2, LN_HALF):
        t = nc.alloc_sbuf_tensor(f"constx{len(nc.const_aps.aps)}", [128, 1], F32)
        nc.gpsimd.memset(t.ap(), v)
        nc.const_aps.aps[(F32, v)] = t.ap()
    nc.all_engine_barrier()

    dram = {}
    for name, shape in [
        ("edata", [128, WPC * C4]), ("deg", [1, APC]), ("scsh", [128, 2 * WPC]),
        ("iota", [128, 128]), ("ident", [128, 128]), ("cent", [128, 5]),
        ("emat", [80, 260]), ("ones", [1, 64]),
        ("wn", [330, 64]), ("wm", [64, 64]),
        ("w1", [64, 512]), ("w2", [512, 512]), ("w3", [512, 1]),
        ("b1", [512, 1]), ("b2", [512, 1]), ("b3", [1, 1]),
    ]:
        dram[name] = nc.dram_tensor(name, shape, F32, kind="ExternalInput").ap()
    d_out = nc.dram_tensor("out", [128, WPC], F32, kind="ExternalOutput").ap()
    d_stage = nc.dram_tensor("stage", [1, APC], F32, kind="Internal").ap()

    with tile.TileContext(nc) as tc:
        from contextlib import ExitStack
        with ExitStack() as ctx:
            P = lambda n, b, **kw: ctx.enter_context(tc.tile_pool(name=n, bufs=b, **kw))
            cpool = P("consts", 1)
            allp = P("allwin", 1)
            spool = P("geoscr", 2)
            fpool = P("feat", 3)
            oipool = P("ohi", 2)
            wpool = P("wmom", 3)     # momB / momuT / momF
            tpool = P("cscr", 2)     # contraction TT scratch
            dpool = P("dft", 3)      # D / Ft
            gtpool = P("gt", 2)
            hpool = P("hmsg", 2)
            apool = P("a1", 1)
            a2pool = P("a2", 1)
            finpool = P("fin", 1)
            pm = P("pm", 2, space="PSUM")
            ptp = P("ptp", 1, space="PSUM")
            pep = P("pep", 2, space="PSUM")
            pmlp = P("pmlp", 2, space="PSUM")
            ph = P("ph", 1, space="PSUM")

            _ld = [0]

            def load(pool, shape, src, dt=F32):
                _ld[0] += 1
                t = pool.tile(list(shape), dt, name=f"ld{_ld[0]}",
                              tag=f"ld{_ld[0]}")
                nc.sync.dma_start(t[:, :], src)
                return t

            iota = load(cpool, [128, 128], dram["iota"][:, :])
            ident = load(cpool, [128, 128], dram["ident"][:, :])
            cent = load(cpool, [128, 5], dram["cent"][:, :])
            onesR = load(cpool, [1, 64], dram["ones"][:, :])
            scsh = load(cpool, [128, 2 * WPC], dram["scsh"][:, :])
            ematF = load(cpool, [80, 260], dram["emat"][:, :])
            wmF = load(cpool, [64, 64], dram["wm"][:, :])
            w1F = load(cpool, [64, 512], dram["w1"][:, :])
            w2F = [load(cpool, [128, 512], dram["w2"][k * 128:(k + 1) * 128, :])
                   for k in range(4)]
            w3F = [load(cpool, [128, 1], dram["w3"][k * 128:(k + 1) * 128, :])
                   for k in range(4)]
            b1 = load(cpool, [128, 4], dram["b1"].rearrange("(m p) q -> p (m q)", p=128))
            b2 = load(cpool, [128, 4], dram["b2"].rearrange("(m p) q -> p (m q)", p=128))
            b3 = load(cpool, [1, 1], dram["b3"][:, :])
            wnF = [load(cpool, [128, 64], dram["wn"][0:128, :]),
                   load(cpool, [128, 64], dram["wn"][128:256, :]),
                   load(cpool, [74, 64], dram["wn"][256:330, :])]
            edata = load(allp, [128, WPC * C4], dram["edata"][:, :])

            # bf16 casts
            def cast(src, shape, name):
                t = cpool.tile(list(shape), BF16, name=name, tag=name)
                nc.vector.tensor_copy(t[:, :], src[:, :])
                return t

            iotaB = cast(iota, [128, 128], "iotaB")
            identB = cast(ident, [128, 128], "identB")
            EB = cast(ematF, [80, 260], "EB")
            wmB = cast(wmF, [64, 64], "wmB")
            w1B = cast(w1F, [64, 512], "w1B")
            w2B = [cast(w2F[k], [128, 512], f"w2B{k}") for k in range(4)]
            w3B = [cast(w3F[k], [128, 1], f"w3B{k}") for k in range(4)]
            wnB = [cast(wnF[0], [128, 64], "wnB0"), cast(wnF[1], [128, 64], "wnB1"),
                   cast(wnF[2], [74, 64], "wnB2")]
            onesB = cast(onesR, [1, 64], "onesB")

            # whole-run tiles
            rAll = allp.tile([128, WPC * C], F32)
            cutAll = allp.tile([128, WPC * C], F32)
            uAll = allp.tile([128, WPC * C3], BF16)
            d5All = allp.tile([128, WPC * C * 5], F32)
            uu6All = allp.tile([128, WPC * C * 6], BF16)
            uuuAll = allp.tile([128, WPC * C * 10], BF16)
            gmAll = allp.tile([128, WPC * 330 + 128], BF16)

            # ---------------- geometry (batched) ----------------
            def emit_geometry(b):
                lo = b * GB
                drv = edata[:, lo * C4:(lo + GB) * C4].rearrange(
                    "p (w q) -> p w q", q=C4)[:, :, 0:C3]
                sq = spool.tile([128, GB * C3], F32, tag="sq")
                nc.scalar.activation(sq[:, :].rearrange("p (w q) -> p w q", q=C3),
                                     drv, ACTF.Square)
                r = rAll[:, lo * C:(lo + GB) * C]
                nc.vector.tensor_reduce(
                    r, sq[:, :].rearrange("p (e x) -> p e x", x=3),
                    axis=AX.X, op=ALU.add)
                nc.scalar.activation(r, r, ACTF.Sqrt)
                rp = spool.tile([128, GB * C], F32, tag="rp")
                nc.vector.tensor_scalar_add(rp[:, :], r, 1e-9)
                nc.vector.reciprocal(rp[:, :], rp[:, :])
                u3 = uAll[:, lo * C3:(lo + GB) * C3].rearrange(
                    "p (w c x) -> p w c x", w=GB, x=3)
                nc.vector.tensor_tensor(
                    u3, drv.rearrange("p w (c x) -> p w c x", x=3),
                    rp[:, :].rearrange("p (w c) -> p w c", w=GB)
                        .unsqueeze(3).broadcast_to((128, GB, C, 3)),
                    ALU.mult)
                nc.vector.tensor_scalar_min(r, r, 6.75)
                sin = spool.tile([128, GB * C], F32, tag="sin")
                nc.scalar.activation(sin[:, :], r, ACTF.Sin,
                                     bias=math.pi / 2, scale=-math.pi / R_MAX)
                m01 = spool.tile([128, GB * C], F32, tag="m01")
                nc.vector.tensor_single_scalar(m01[:, :], r, R_MAX, ALU.is_lt)
                cut = cutAll[:, lo * C:(lo + GB) * C]
                nc.vector.scalar_tensor_tensor(
                    cut, sin[:, :], 1.0, m01[:, :], ALU.add, ALU.mult)
                d5 = d5All[:, lo * C * 5:(lo + GB) * C * 5].rearrange(
                    "p (e k) -> p e k", k=5)
                nc.vector.tensor_tensor(
                    d5, r.unsqueeze(2).broadcast_to((128, GB * C, 5)),
                    cent[:, :].unsqueeze(1).broadcast_to((128, GB * C, 5)),
                    ALU.subtract)
                nc.scalar.activation(d5, d5, ACTF.Square)
                nc.scalar.activation(d5, d5, ACTF.Exp, bias=LN_HALF, scale=-1.0)
                nc.vector.tensor_tensor(
                    d5, d5, cut.unsqueeze(2).broadcast_to((128, GB * C, 5)),
                    ALU.mult)
                # unit-vector monomials (batched): uu6 pairs, uuu10 triples
                E_ = GB * C
                ub = u3.rearrange("p w c x -> p (w c) x")
                uu = uu6All[:, lo * C * 6:(lo + GB) * C * 6].rearrange(
                    "p (e q) -> p e q", q=6)
                nc.vector.tensor_tensor(
                    uu[:, :, 0:3], ub[:, :, 0:1].broadcast_to((128, E_, 3)),
                    ub[:, :, 0:3], ALU.mult)
                nc.vector.tensor_tensor(
                    uu[:, :, 3:5], ub[:, :, 1:2].broadcast_to((128, E_, 2)),
                    ub[:, :, 1:3], ALU.mult)
                nc.vector.tensor_tensor(
                    uu[:, :, 5:6], ub[:, :, 2:3], ub[:, :, 2:3], ALU.mult)
                uv = uuuAll[:, lo * C * 10:(lo + GB) * C * 10].rearrange(
                    "p (e q) -> p e q", q=10)
                nc.vector.tensor_tensor(
                    uv[:, :, 0:6], ub[:, :, 0:1].broadcast_to((128, E_, 6)),
                    uu[:, :, 0:6], ALU.mult)
                nc.vector.tensor_tensor(
                    uv[:, :, 6:9], ub[:, :, 1:2].broadcast_to((128, E_, 3)),
                    uu[:, :, 3:6], ALU.mult)
                nc.vector.tensor_tensor(
                    uv[:, :, 9:10], ub[:, :, 2:3], uu[:, :, 5:6], ALU.mult)

            # ---------------- per-window loop ----------------
            def emit_window(w):
                u3 = uAll[:, w * C3:(w + 1) * C3].rearrange("p (c x) -> p c x", x=3)
                d5w = d5All[:, w * C * 5:(w + 1) * C * 5].rearrange(
                    "p (c k) -> p c k", k=5)
                uu = uu6All[:, w * C * 6:(w + 1) * C * 6].rearrange(
                    "p (c q) -> p c q", q=6)
                uv = uuuAll[:, w * C * 10:(w + 1) * C * 10].rearrange(
                    "p (c q) -> p c q", q=10)

                F = fpool.tile([128, C * 100], BF16, tag="F")
                Fv = F[:, :].rearrange("p (c f) -> p c f", f=100)
                nc.scalar.copy(Fv[:, :, 0:5], d5w)
                nc.vector.tensor_tensor(
                    Fv[:, :, 5:20].rearrange("p c (r x) -> p c r x", x=3),
                    d5w.unsqueeze(3).broadcast_to((128, C, 5, 3)),
                    u3.unsqueeze(2).broadcast_to((128, C, 5, 3)), ALU.mult)
                nc.vector.tensor_tensor(
                    Fv[:, :, 20:50].rearrange("p c (r q) -> p c r q", q=6),
                    d5w.unsqueeze(3).broadcast_to((128, C, 5, 6)),
                    uu.unsqueeze(2).broadcast_to((128, C, 5, 6)), ALU.mult)
                nc.gpsimd.tensor_tensor(
                    Fv[:, :, 50:100].rearrange("p c (r q) -> p c r q", q=10),
                    d5w.unsqueeze(3).broadcast_to((128, C, 5, 10)),
                    uv.unsqueeze(2).broadcast_to((128, C, 5, 10)), ALU.mult)

                ohi = oipool.tile([128, C * 128], BF16, tag="ohi")
                for c in range(C):
                    eng = nc.gpsimd if c >= C - 3 else nc.vector
                    eng.tensor_scalar(
                        ohi[:, c * 128:(c + 1) * 128], iotaB[:, :],
                        edata[:, w * C4 + C3 + c:w * C4 + C3 + c + 1], None,
                        ALU.is_equal)

                mom_ps = pm.tile([128, 100], F32)
                for c in range(C):
                    nc.tensor.matmul(mom_ps[:, :], ohi[:, c * 128:(c + 1) * 128],
                                     F[:, c * 100:(c + 1) * 100],
                                     start=(c == 0), stop=(c == C - 1))
                momB = wpool.tile([128, 100], BF16, tag="momB")
                nc.scalar.activation(momB[:, :], mom_ps[:, :], ACTF.Copy)

                gm = gmAll[:, w * 330:w * 330 + 330]
                nc.scalar.copy(gm[:, 0:5], momB[:, 0:5])

                # expand unique moments -> full layouts via PE
                ptT = ptp.tile([80, 128], BF16)
                nc.tensor.matmul(ptT[0:80, 0:128], momB[:, 20:100], identB[:, :],
                                 is_transpose=True, start=True, stop=True)
                momuT = wpool.tile([80, 128], BF16, tag="momuT")
                nc.vector.tensor_copy(momuT[:, :], ptT[0:80, 0:128])
                pe_ps = pep.tile([128, 260], F32)
                nc.tensor.matmul(pe_ps[:, :], momuT[:, :], EB[:, :],
                                 start=True, stop=True)
                momF = wpool.tile([128, 260], BF16, tag="momF")
                nc.scalar.activation(momF[:, :], pe_ps[:, :], ACTF.Copy)

                m1 = momB[:, 5:20].rearrange("p (r i) -> p r i", r=5)
                m2u = momB[:, 20:50].rearrange("p (r q) -> p r q", r=5)
                m3u = momB[:, 50:100].rearrange("p (r q) -> p r q", r=5)
                m2f = momF[:, 0:45].rearrange("p (r j i) -> p r j i", r=5, j=3)
                m2f9 = momF[:, 0:45].rearrange("p (r q) -> p r q", r=5)
                m3k = momF[:, 45:180].rearrange("p (r k q) -> p r k q", r=5, k=3)
                m2w = momF[:, 180:210].rearrange("p (r q) -> p r q", r=5)
                m3w = momF[:, 210:260].rearrange("p (r q) -> p r q", r=5)

                with nc.allow_low_precision(reason="bf16 gm"):
                    t1 = tpool.tile([128, 75], BF16, tag="t1")
                    nc.vector.tensor_tensor(
                        t1[:, :].rearrange("p (r s i) -> p r s i", r=5, s=5),
                        m1.unsqueeze(2).broadcast_to((128, 5, 5, 3)),
                        m1.unsqueeze(1).broadcast_to((128, 5, 5, 3)), ALU.mult)
                    nc.vector.tensor_reduce(
                        gm[:, 5:30], t1[:, :].rearrange("p (q i) -> p q i", i=3),
                        axis=AX.X, op=ALU.add)
                    t2 = tpool.tile([128, 150], BF16, tag="t2")
                    nc.vector.tensor_tensor(
                        t2[:, :].rearrange("p (r s q) -> p r s q", r=5, s=5),
                        m2u.unsqueeze(2).broadcast_to((128, 5, 5, 6)),
                        m2w.unsqueeze(1).broadcast_to((128, 5, 5, 6)), ALU.mult)
                    nc.vector.tensor_reduce(
                        gm[:, 30:55], t2[:, :].rearrange("p (q x) -> p q x", x=6),
                        axis=AX.X, op=ALU.add)
                    t3 = tpool.tile([128, 250], BF16, tag="t3")
                    nc.vector.tensor_tensor(
                        t3[:, :].rearrange("p (r s q) -> p r s q", r=5, s=5),
                        m3u.unsqueeze(2).broadcast_to((128, 5, 5, 10)),
                        m3w.unsqueeze(1).broadcast_to((128, 5, 5, 10)), ALU.mult)
                    nc.vector.tensor_reduce(
                        gm[:, 55:80], t3[:, :].rearrange("p (q x) -> p q x", x=10),
                        axis=AX.X, op=ALU.add)
                    tD = tpool.tile([128, 225], BF16, tag="tD")
                    nc.vector.tensor_tensor(
                        tD[:, :].rearrange("p (r j s i) -> p r j s i",
                                           r=5, j=3, s=5),
                        m2f.unsqueeze(3).broadcast_to((128, 5, 3, 5, 3)),
                        m1.unsqueeze(1).unsqueeze(1)
                            .broadcast_to((128, 5, 3, 5, 3)), ALU.mult)
                    D = dpool.tile([128, 75], BF16, tag="D")  # memory [r,s,j]
                    nc.vector.tensor_reduce(
                        D[:, :].rearrange("p (r s j) -> p r j s", r=5, s=5),
                        tD[:, :].rearrange("p (q i) -> p q i", i=3),
                        axis=AX.X, op=ALU.add)
                    t4 = tpool.tile([128, 375], BF16, tag="t4")
                    nc.vector.tensor_tensor(
                        t4[:, :].rearrange("p (r s t j) -> p r s t j",
                                               r=5, s=5, t=5),
                        D[:, :].rearrange("p (r s j) -> p r s j", r=5, s=5)
                            .unsqueeze(3).broadcast_to((128, 5, 5, 5, 3)),
                        m1.unsqueeze(1).unsqueeze(1)
                            .broadcast_to((128, 5, 5, 5, 3)), ALU.mult)
                    nc.vector.tensor_reduce(
                        gm[:, 80:205], t4[:, :].rearrange("p (q j) -> p q j", j=3),
                        axis=AX.X, op=ALU.add)
                    tF = tpool.tile([128, 675], BF16, tag="tF")
                    nc.gpsimd.tensor_tensor(
                        tF[:, :].rearrange("p (r k s q) -> p r k s q",
                                           r=5, k=3, s=5),
                        m3k.unsqueeze(3).broadcast_to((128, 5, 3, 5, 9)),
                        m2f9.unsqueeze(1).unsqueeze(1)
                            .broadcast_to((128, 5, 3, 5, 9)),
                        ALU.mult)
                    Ft = dpool.tile([128, 75], BF16, tag="Ft")  # memory [r,s,k]
                    nc.vector.tensor_reduce(
                        Ft[:, :].rearrange("p (r s k) -> p r k s", r=5, s=5),
                        tF[:, :].rearrange("p (q x) -> p q x", x=9),
                        axis=AX.X, op=ALU.add)
                    t5 = tpool.tile([128, 375], BF16, tag="t5")
                    nc.gpsimd.tensor_tensor(
                        t5[:, :].rearrange("p (r s t k) -> p r s t k",
                                               r=5, s=5, t=5),
                        Ft[:, :].rearrange("p (r s k) -> p r s k", r=5, s=5)
                            .unsqueeze(3).broadcast_to((128, 5, 5, 5, 3)),
                        m1.unsqueeze(1).unsqueeze(1)
                            .broadcast_to((128, 5, 5, 5, 3)), ALU.mult)
                    nc.vector.tensor_reduce(
                        gm[:, 205:330], t5[:, :].rearrange("p (q k) -> p q k", k=3),
                        axis=AX.X, op=ALU.add)

            # ---------------- readout MLP per group ----------------
            def emit_group(g):
                w = 4 * g
                nw = min(4, WPC - w)
                na = nw * 128
                gt = [gtpool.tile([128, 512], BF16, tag=f"gt{k}", name=f"gt{k}")
                      for k in range(3)]
                for wi in range(nw):
                    base = (w + wi) * 330
                    nc.sync.dma_start_transpose(
                        gt[0][0:128, wi * 128:(wi + 1) * 128],
                        gmAll[:, base:base + 128])
                    nc.sync.dma_start_transpose(
                        gt[1][0:128, wi * 128:(wi + 1) * 128],
                        gmAll[:, base + 128:base + 256])
                    nc.sync.dma_start_transpose(
                        gt[2][0:128, wi * 128:(wi + 1) * 128],
                        gmAll[:, base + 256:base + 384])
                hps = ph.tile([64, 512], F32, tag="hps")
                nc.tensor.matmul(hps[:, 0:na], wnB[0][:, :], gt[0][:, 0:na],
                                 start=True, stop=False)
                nc.tensor.matmul(hps[:, 0:na], wnB[1][:, :], gt[1][:, 0:na],
                                 start=False, stop=False)
                nc.tensor.matmul(hps[:, 0:na], wnB[2][:, :], gt[2][0:74, 0:na],
                                 start=False, stop=True)
                hT = hpool.tile([64, 512], BF16, tag="hT")
                nc.scalar.activation(hT[:, 0:na], hps[:, 0:na], ACTF.Identity,
                                     scale=S330)
                mps = ph.tile([64, 512], F32, tag="hps")
                nc.tensor.matmul(mps[:, 0:na], wmB[:, :], hT[:, 0:na],
                                 start=True, stop=True)
                msgT = hpool.tile([64, 512], BF16, tag="msgT")
                nc.scalar.activation(msgT[:, 0:na], mps[:, 0:na], ACTF.Silu,
                                     scale=S64)
                degt = hpool.tile([1, 512], F32, tag="degt")
                nc.sync.dma_start(degt[0:1, 0:na],
                                  dram["deg"][0:1, w * 128:w * 128 + na])
                degtB = hpool.tile([1, 512], BF16, tag="degtB")
                nc.scalar.copy(degtB[0:1, 0:na], degt[0:1, 0:na])
                dps = ph.tile([64, 512], F32, tag="hps")
                nc.tensor.matmul(dps[:, 0:na], onesB[:, :],
                                 degtB[0:1, 0:na], start=True, stop=True)
                msgd = hpool.tile([64, 512], BF16, tag="msgd")
                nc.vector.tensor_tensor(msgd[:, 0:na], msgT[:, 0:na],
                                        dps[:, 0:na], ALU.mult)
                hpT = hpool.tile([64, 512], BF16, tag="hpT")
                nc.vector.tensor_tensor(hpT[:, 0:na], hT[:, 0:na],
                                        msgd[:, 0:na], ALU.add)
                a1 = [apool.tile([128, 512], BF16, tag=f"a1_{m}", name=f"a1_{m}")
                      for m in range(4)]
                for m in range(4):
                    ps = pmlp.tile([128, 512], F32)
                    nc.tensor.matmul(ps[:, 0:na], w1B[:, m * 128:(m + 1) * 128],
                                     hpT[:, 0:na], start=True, stop=True)
                    nc.scalar.activation(a1[m][:, 0:na], ps[:, 0:na], ACTF.Silu,
                                         bias=b1[:, m:m + 1], scale=S64)
                a2 = [a2pool.tile([128, 512], BF16, tag=f"a2_{m}", name=f"a2_{m}")
                      for m in range(4)]
                for m in range(4):
                    ps = pmlp.tile([128, 512], F32)
                    for k in range(4):
                        nc.tensor.matmul(ps[:, 0:na],
                                         w2B[k][:, m * 128:(m + 1) * 128],
                                         a1[k][:, 0:na],
                                         start=(k == 0), stop=(k == 3))
                    nc.scalar.activation(a2[m][:, 0:na], ps[:, 0:na], ACTF.Silu,
                                         bias=b2[:, m:m + 1], scale=S512)
                ops_ = ph.tile([64, 512], F32, tag="hps")
                for k in range(4):
                    nc.tensor.matmul(ops_[0:1, 0:na], w3B[k][:, :], a2[k][:, 0:na],
                                     start=(k == 0), stop=(k == 3))
                orow = hpool.tile([1, 512], F32, tag="orow")
                nc.scalar.activation(orow[0:1, 0:na], ops_[0:1, 0:na],
                                     ACTF.Identity, bias=b3[0:1, 0:1], scale=S512)
                nc.sync.dma_start(d_stage[0:1, w * 128:w * 128 + na],
                                  orow[0:1, 0:na])

            # ---------------- interleaved emission ----------------
            for b in range(NB):
                emit_geometry(b)
                for w in range(b * GB, (b + 1) * GB):
                    emit_window(w)
                    if (w + 1) % 4 == 0:
                        emit_group((w + 1) // 4 - 1)
            emit_group(12)

            # ---------------- epilogue ----------------
            outM = finpool.tile([128, WPC], F32)
            nc.sync.dma_start(outM[:, :],
                              d_stage.rearrange("q (w p) -> (q p) w", p=128))
            nc.vector.tensor_tensor(outM[:, :], outM[:, :], scsh[:, 0:WPC],
                                    ALU.mult)
            nc.vector.tensor_tensor(outM[:, :], outM[:, :], scsh[:, WPC:2 * WPC],
                                    ALU.add)
            nc.sync.dma_start(d_out[:, :], outM[:, :])

    nc.compile()
    return nc


_CACHED = {}


def kernel(dr_vec, Z, idx, W_node, W_msg, W_r1, b_r1, W_r2, b_r2, W_r3, b_r3,
           scale, shift):
    dr_vec = np.asarray(dr_vec, np.float32)
    Z = np.asarray(Z).astype(np.int64)
    i_idx = np.asarray(idx[0], np.int64)
    j_idx = np.asarray(idx[1], np.int64)

    C = int(np.ceil(np.bincount(i_idx >> 7, minlength=400).max() / 128))
    C3, C4 = 3 * C, 4 * C

    drP, irelP = _pack_edges(i_idx, dr_vec, C)

    deg = np.bincount(j_idx, minlength=N_PAD).astype(np.float32)

    mask = (Z != 0).astype(np.float32)
    scz = np.zeros(N_PAD, np.float32)
    shz = np.zeros(N_PAD, np.float32)
    scz[:N_ATOMS] = np.asarray(scale, np.float32).ravel()[Z] * mask
    shz[:N_ATOMS] = np.asarray(shift, np.float32).ravel()[Z] * mask

    common = dict(
        iota=np.broadcast_to(np.arange(128, dtype=np.float32)[None, :],
                             (128, 128)).copy(),
        ident=np.eye(128, dtype=np.float32),
        cent=np.broadcast_to(np.linspace(0.5, R_MAX, 5, dtype=np.float32)[None, :],
                             (128, 5)).copy(),
        ones=np.ones((1, 64), np.float32),
        emat=_build_E(),
        wn=np.asarray(W_node, np.float32), wm=np.asarray(W_msg, np.float32),
        w1=np.asarray(W_r1, np.float32), w2=np.asarray(W_r2, np.float32),
        w3=np.asarray(W_r3, np.float32),
        b1=np.asarray(b_r1, np.float32).reshape(512, 1),
        b2=np.asarray(b_r2, np.float32).reshape(512, 1),
        b3=np.asarray(b_r3, np.float32).reshape(1, 1),
    )
    in_maps = []
    for cc in range(N_CORES):
        w0 = cc * WPC
        m = dict(common)
        # edata: [128, WPC*C4] partition-major; per window [dr 3C | irel C]
        ed = np.empty((WPC, 128, C4), np.float32)
        ed[:, :, 0:C3] = drP[w0:w0 + WPC].reshape(WPC, 128, C3)
        ed[:, :, C3:C4] = irelP[w0:w0 + WPC]
        m["edata"] = np.ascontiguousarray(ed.transpose(1, 0, 2)).reshape(128, WPC * C4)
        m["deg"] = deg[cc * APC:(cc + 1) * APC].reshape(1, APC)
        sc = scz[cc * APC:(cc + 1) * APC].reshape(WPC, 128).T
        sh = shz[cc * APC:(cc + 1) * APC].reshape(WPC, 128).T
        m["scsh"] = np.ascontiguousarray(np.concatenate([sc, sh], axis=1))
        in_maps.append(m)

    key = (C,)
    if key not in _CACHED:
        _CACHED[key] = _build_program(C)
    nc = _CACHED[key]

    import time as _t
    t0 = _t.time()
    res = bass_utils.run_bass_kernel_spmd(
        nc, in_maps, core_ids=list(range(N_CORES)))
    t1 = _t.time()
    if os.environ.get("BENCH_TIME"):
        print(f"device run wall: {(t1 - t0) * 1e3:.1f} ms")
    if res.exec_time_ns is not None:
        print(f"HW exec time: {res.exec_time_ns} ns")
    outs = [res.results[c]["out"].T.ravel() for c in range(N_CORES)]
    full = np.concatenate(outs)[:N_ATOMS].astype(np.float32)
    return full[:, None]


# revision 26
# speedup vs baseline: 2.4529x; 1.0371x over previous
import os, sys, math
import numpy as np

sys.path.insert(0, "/opt/trn_rl_repo")

import concourse.bass as bass
import concourse.bacc as bacc
import concourse.mybir as mybir
from concourse import tile
from concourse import bass_utils

F32 = mybir.dt.float32
BF16 = mybir.dt.bfloat16
F32R = mybir.dt.float32r
ALU = mybir.AluOpType
ACTF = mybir.ActivationFunctionType
AX = mybir.AxisListType

N_ATOMS = 50000
N_CORES = 8
N_PAD = 51200            # 8 * 6400
APC = 6400               # atoms per core
WPC = 50                 # 128-atom windows per core
GB = 10                  # windows per geometry batch
R_MAX = 6.0
LN_HALF = math.log(0.5)
S330 = 1.0 / math.sqrt(330.0)
S64 = 1.0 / math.sqrt(64.0)
S512 = 1.0 / math.sqrt(512.0)

# unique index orders
PAIRS = [(0, 0), (0, 1), (0, 2), (1, 1), (1, 2), (2, 2)]
TRIPS = [(0, 0, 0), (0, 0, 1), (0, 0, 2), (0, 1, 1), (0, 1, 2),
         (0, 2, 2), (1, 1, 1), (1, 1, 2), (1, 2, 2), (2, 2, 2)]
PAIR_IDX = {p: k for k, p in enumerate(PAIRS)}
TRIP_IDX = {t: k for k, t in enumerate(TRIPS)}
W6 = np.array([1, 2, 2, 1, 2, 1], np.float32)
W10 = np.array([1, 3, 3, 3, 6, 3, 1, 3, 3, 1], np.float32)


def _pack_edges(key_idx, dr, C):
    """Sort edges by center atom, bucket into 128-atom windows, pad each
    window to C chunks of 128."""
    order = np.argsort(key_idx, kind="stable")
    k_s = key_idx[order]
    win = (k_s >> 7).astype(np.int64)
    cnt = np.bincount(win, minlength=400)
    start = np.zeros(400, np.int64)
    start[1:] = np.cumsum(cnt)[:-1]
    rank = np.arange(len(k_s)) - start[win]
    p = rank % 128
    c = rank // 128
    relP = np.zeros((400, 128, C), np.float32)
    relP[win, p, c] = (k_s & 127).astype(np.float32)
    drP = np.zeros((400, 128, C, 3), np.float32)
    drP[..., 0] = 100.0
    drP[win, p, c] = dr[order]
    return drP, relP


def _build_E():
    """[80, 260] expansion matrix: momuT rows (r,p6 | r,t10) ->
    [m2f 45 (r,j,i) | m3kij 135 (r,k,ij) | m2uw 30 | m3uw 50]."""
    E = np.zeros((80, 260), np.float32)
    for r in range(5):
        for j in range(3):
            for i in range(3):
                E[6 * r + PAIR_IDX[tuple(sorted((j, i)))], 9 * r + 3 * j + i] = 1.0
        for k in range(3):
            for i in range(3):
                for j in range(3):
                    t = TRIP_IDX[tuple(sorted((i, j, k)))]
                    E[30 + 10 * r + t, 45 + 27 * r + 9 * k + 3 * i + j] = 1.0
        for p in range(6):
            E[6 * r + p, 180 + 6 * r + p] = W6[p]
        for t in range(10):
            E[30 + 10 * r + t, 210 + 10 * r + t] = W10[t]
    return E


def _build_program(C):
    C3 = 3 * C
    C4 = 4 * C
    NB = WPC // GB
    nc = bacc.Bacc("TRN2", target_bir_lowering=False, debug=False)

    for v in (math.pi / 2, LN_HALF):
        t = nc.alloc_sbuf_tensor(f"constx{len(nc.const_aps.aps)}", [128, 1], F32)
        nc.gpsimd.memset(t.ap(), v)
        nc.const_aps.aps[(F32, v)] = t.ap()
    nc.all_engine_barrier()

    dram = {}
    for name, shape in [
        ("edata", [128, WPC * C4]), ("deg", [1, APC]), ("scsh", [128, 2 * WPC]),
        ("iota", [128, 128]), ("ident", [128, 128]), ("cent", [128, 5]),
        ("emat", [80, 260]), ("ones", [1, 64]),
        ("wn", [330, 64]), ("wm", [64, 64]),
        ("w1", [64, 512]), ("w2", [512, 512]), ("w3", [512, 1]),
        ("b1", [512, 1]), ("b2", [512, 1]), ("b3", [1, 1]),
    ]:
        dram[name] = nc.dram_tensor(name, shape, F32, kind="ExternalInput").ap()
    d_out = nc.dram_tensor("out", [128, WPC], F32, kind="ExternalOutput").ap()
    d_stage = nc.dram_tensor("stage", [1, APC], F32, kind="Internal").ap()

    with tile.TileContext(nc) as tc:
        from contextlib import ExitStack
        with ExitStack() as ctx:
            P = lambda n, b, **kw: ctx.enter_context(tc.tile_pool(name=n, bufs=b, **kw))
            cpool = P("consts", 1)
            allp = P("allwin", 1)
            spool = P("geoscr", 1)
            fpool = P("feat", 3)
            oipool = P("ohi", 2)
            wpool = P("wmom", 3)     # momB / momuT / momF
            tpool = P("cscr", 2)     # contraction TT scratch
            dpool = P("dft", 3)      # D / Ft
            gtpool = P("gt", 2)
            hpool = P("hmsg", 2)
            apool = P("a1", 1)
            a2pool = P("a2", 1)
            finpool = P("fin", 1)
            pm = P("pm", 2, space="PSUM")
            ptp = P("ptp", 1, space="PSUM")
            pep = P("pep", 2, space="PSUM")
            pmlp = P("pmlp", 2, space="PSUM")
            ph = P("ph", 1, space="PSUM")

            _ld = [0]

            def load(pool, shape, src, dt=F32, tag=None):
                _ld[0] += 1
                t = pool.tile(list(shape), dt, name=f"ld{_ld[0]}",
                              tag=tag or f"ld{_ld[0]}")
                nc.sync.dma_start(t[:, :], src)
                return t

            def loadtmp(shape, src):
                return load(spool, shape, src, tag="wtmp")

            edata = load(allp, [128, WPC * C4], dram["edata"][:, :])
            iota = loadtmp([128, 128], dram["iota"][:, :])
            ident = loadtmp([128, 128], dram["ident"][:, :])
            cent = load(cpool, [128, 5], dram["cent"][:, :])
            onesR = load(cpool, [1, 64], dram["ones"][:, :])
            scsh = load(cpool, [128, 2 * WPC], dram["scsh"][:, :])
            ematF = loadtmp([80, 260], dram["emat"][:, :])
            wmF = loadtmp([64, 64], dram["wm"][:, :])
            w1F = loadtmp([64, 512], dram["w1"][:, :])
            w2F = [loadtmp([128, 512], dram["w2"][k * 128:(k + 1) * 128, :])
                   for k in range(4)]
            w3F = [loadtmp([128, 1], dram["w3"][k * 128:(k + 1) * 128, :])
                   for k in range(4)]
            b1 = load(cpool, [128, 4], dram["b1"].rearrange("(m p) q -> p (m q)", p=128))
            b2 = load(cpool, [128, 4], dram["b2"].rearrange("(m p) q -> p (m q)", p=128))
            b3 = load(cpool, [1, 1], dram["b3"][:, :])
            wnF = [loadtmp([128, 64], dram["wn"][0:128, :]),
                   loadtmp([128, 64], dram["wn"][128:256, :]),
                   loadtmp([74, 64], dram["wn"][256:330, :])]

            # dtype casts
            def cast(src, shape, name, dt=BF16):
                t = cpool.tile(list(shape), dt, name=name, tag=name)
                nc.vector.tensor_copy(t[:, :], src[:, :])
                return t

            iotaB = cast(iota, [128, 128], "iotaB")
            identB = cast(ident, [128, 128], "identB")
            EB = cast(ematF, [80, 260], "EB")
            wmR = cast(wmF, [64, 64], "wmR", F32R)
            w1R = cast(w1F, [64, 512], "w1R", F32R)
            w2R = [cast(w2F[k], [128, 512], f"w2R{k}", F32R) for k in range(4)]
            w3R = [cast(w3F[k], [128, 1], f"w3R{k}", F32R) for k in range(4)]
            wnB = [cast(wnF[0], [128, 64], "wnB0"), cast(wnF[1], [128, 64], "wnB1"),
                   cast(wnF[2], [74, 64], "wnB2")]
            onesR32 = cast(onesR, [1, 64], "onesR32", F32R)

            # whole-run tiles
            rAll = allp.tile([128, WPC * C], F32)
            cutAll = allp.tile([128, WPC * C], F32)
            uAll = allp.tile([128, WPC * C3], BF16)
            d5All = allp.tile([128, WPC * C * 5], BF16)
            uu6All = allp.tile([128, WPC * C * 6], BF16)
            uuuAll = allp.tile([128, WPC * C * 10], BF16)
            gmAll = allp.tile([128, WPC * 330 + 128], BF16)

            # ---------------- geometry (batched) ----------------
            def emit_geometry(b):
                lo = b * GB
                drv = edata[:, lo * C4:(lo + GB) * C4].rearrange(
                    "p (w q) -> p w q", q=C4)[:, :, 0:C3]
                sq = spool.tile([128, GB * C3], F32, tag="sq")
                nc.scalar.activation(sq[:, :].rearrange("p (w q) -> p w q", q=C3),
                                     drv, ACTF.Square)
                r = rAll[:, lo * C:(lo + GB) * C]
                nc.vector.tensor_reduce(
                    r, sq[:, :].rearrange("p (e x) -> p e x", x=3),
                    axis=AX.X, op=ALU.add)
                nc.scalar.activation(r, r, ACTF.Sqrt)
                rp = spool.tile([128, GB * C], F32, tag="rp")
                nc.vector.tensor_scalar_add(rp[:, :], r, 1e-9)
                nc.vector.reciprocal(rp[:, :], rp[:, :])
                u3 = uAll[:, lo * C3:(lo + GB) * C3].rearrange(
                    "p (w c x) -> p w c x", w=GB, x=3)
                nc.vector.tensor_tensor(
                    u3, drv.rearrange("p w (c x) -> p w c x", x=3),
                    rp[:, :].rearrange("p (w c) -> p w c", w=GB)
                        .unsqueeze(3).broadcast_to((128, GB, C, 3)),
                    ALU.mult)
                nc.vector.tensor_scalar_min(r, r, 6.75)
                sin = spool.tile([128, GB * C], F32, tag="sin")
                nc.scalar.activation(sin[:, :], r, ACTF.Sin,
                                     bias=math.pi / 2, scale=-math.pi / R_MAX)
                m01 = spool.tile([128, GB * C], F32, tag="m01")
                nc.vector.tensor_single_scalar(m01[:, :], r, R_MAX, ALU.is_lt)
                cut = cutAll[:, lo * C:(lo + GB) * C]
                nc.vector.scalar_tensor_tensor(
                    cut, sin[:, :], 1.0, m01[:, :], ALU.add, ALU.mult)
                d5 = d5All[:, lo * C * 5:(lo + GB) * C * 5].rearrange(
                    "p (e k) -> p e k", k=5)
                nc.vector.tensor_tensor(
                    d5, r.unsqueeze(2).broadcast_to((128, GB * C, 5)),
                    cent[:, :].unsqueeze(1).broadcast_to((128, GB * C, 5)),
                    ALU.subtract)
                nc.scalar.activation(d5, d5, ACTF.Square)
                nc.scalar.activation(d5, d5, ACTF.Exp, bias=LN_HALF, scale=-1.0)
                nc.vector.tensor_tensor(
                    d5, d5, cut.unsqueeze(2).broadcast_to((128, GB * C, 5)),
                    ALU.mult)
                # unit-vector monomials (batched): uu6 pairs, uuu10 triples
                E_ = GB * C
                ub = u3.rearrange("p w c x -> p (w c) x")
                uu = uu6All[:, lo * C * 6:(lo + GB) * C * 6].rearrange(
                    "p (e q) -> p e q", q=6)
                nc.vector.tensor_tensor(
                    uu[:, :, 0:3], ub[:, :, 0:1].broadcast_to((128, E_, 3)),
                    ub[:, :, 0:3], ALU.mult)
                nc.vector.tensor_tensor(
                    uu[:, :, 3:5], ub[:, :, 1:2].broadcast_to((128, E_, 2)),
                    ub[:, :, 1:3], ALU.mult)
                nc.vector.tensor_tensor(
                    uu[:, :, 5:6], ub[:, :, 2:3], ub[:, :, 2:3], ALU.mult)
                uv = uuuAll[:, lo * C * 10:(lo + GB) * C * 10].rearrange(
                    "p (e q) -> p e q", q=10)
                nc.vector.tensor_tensor(
                    uv[:, :, 0:6], ub[:, :, 0:1].broadcast_to((128, E_, 6)),
                    uu[:, :, 0:6], ALU.mult)
                nc.vector.tensor_tensor(
                    uv[:, :, 6:9], ub[:, :, 1:2].broadcast_to((128, E_, 3)),
                    uu[:, :, 3:6], ALU.mult)
                nc.vector.tensor_tensor(
                    uv[:, :, 9:10], ub[:, :, 2:3], uu[:, :, 5:6], ALU.mult)

            # ---------------- per-window loop ----------------
            def emit_window(w):
                u3 = uAll[:, w * C3:(w + 1) * C3].rearrange("p (c x) -> p c x", x=3)
                d5w = d5All[:, w * C * 5:(w + 1) * C * 5].rearrange(
                    "p (c k) -> p c k", k=5)
                uu = uu6All[:, w * C * 6:(w + 1) * C * 6].rearrange(
                    "p (c q) -> p c q", q=6)
                uv = uuuAll[:, w * C * 10:(w + 1) * C * 10].rearrange(
                    "p (c q) -> p c q", q=10)

                F = fpool.tile([128, C * 100], BF16, tag="F")
                Fv = F[:, :].rearrange("p (c f) -> p c f", f=100)
                nc.scalar.copy(Fv[:, :, 0:5], d5w)
                nc.vector.tensor_tensor(
                    Fv[:, :, 5:20].rearrange("p c (r x) -> p c r x", x=3),
                    d5w.unsqueeze(3).broadcast_to((128, C, 5, 3)),
                    u3.unsqueeze(2).broadcast_to((128, C, 5, 3)), ALU.mult)
                nc.vector.tensor_tensor(
                    Fv[:, :, 20:50].rearrange("p c (r q) -> p c r q", q=6),
                    d5w.unsqueeze(3).broadcast_to((128, C, 5, 6)),
                    uu.unsqueeze(2).broadcast_to((128, C, 5, 6)), ALU.mult)
                nc.gpsimd.tensor_tensor(
                    Fv[:, :, 50:100].rearrange("p c (r q) -> p c r q", q=10),
                    d5w.unsqueeze(3).broadcast_to((128, C, 5, 10)),
                    uv.unsqueeze(2).broadcast_to((128, C, 5, 10)), ALU.mult)

                ohi = oipool.tile([128, C * 128], BF16, tag="ohi")
                for c in range(C):
                    eng = nc.gpsimd if c >= C - 3 else nc.vector
                    eng.tensor_scalar(
                        ohi[:, c * 128:(c + 1) * 128], iotaB[:, :],
                        edata[:, w * C4 + C3 + c:w * C4 + C3 + c + 1], None,
                        ALU.is_equal)

                mom_ps = pm.tile([128, 100], F32)
                for c in range(C):
                    nc.tensor.matmul(mom_ps[:, :], ohi[:, c * 128:(c + 1) * 128],
                                     F[:, c * 100:(c + 1) * 100],
                                     start=(c == 0), stop=(c == C - 1))
                momB = wpool.tile([128, 100], BF16, tag="momB")
                nc.scalar.activation(momB[:, :], mom_ps[:, :], ACTF.Copy)

                gm = gmAll[:, w * 330:w * 330 + 330]
                nc.scalar.copy(gm[:, 0:5], momB[:, 0:5])

                # expand unique moments -> full layouts via PE
                ptT = ptp.tile([80, 128], BF16)
                nc.tensor.matmul(ptT[0:80, 0:128], momB[:, 20:100], identB[:, :],
                                 is_transpose=True, start=True, stop=True)
                momuT = wpool.tile([80, 128], BF16, tag="momuT")
                nc.vector.tensor_copy(momuT[:, :], ptT[0:80, 0:128])
                pe_ps = pep.tile([128, 260], F32)
                nc.tensor.matmul(pe_ps[:, :], momuT[:, :], EB[:, :],
                                 start=True, stop=True)
                momF = wpool.tile([128, 260], BF16, tag="momF")
                nc.scalar.activation(momF[:, :], pe_ps[:, :], ACTF.Copy)

                m1 = momB[:, 5:20].rearrange("p (r i) -> p r i", r=5)
                m2u = momB[:, 20:50].rearrange("p (r q) -> p r q", r=5)
                m3u = momB[:, 50:100].rearrange("p (r q) -> p r q", r=5)
                m2f = momF[:, 0:45].rearrange("p (r j i) -> p r j i", r=5, j=3)
                m2f9 = momF[:, 0:45].rearrange("p (r q) -> p r q", r=5)
                m3k = momF[:, 45:180].rearrange("p (r k q) -> p r k q", r=5, k=3)
                m2w = momF[:, 180:210].rearrange("p (r q) -> p r q", r=5)
                m3w = momF[:, 210:260].rearrange("p (r q) -> p r q", r=5)

                with nc.allow_low_precision(reason="bf16 gm"):
                    t1 = tpool.tile([128, 75], BF16, tag="t1")
                    nc.vector.tensor_tensor(
                        t1[:, :].rearrange("p (r s i) -> p r s i", r=5, s=5),
                        m1.unsqueeze(2).broadcast_to((128, 5, 5, 3)),
                        m1.unsqueeze(1).broadcast_to((128, 5, 5, 3)), ALU.mult)
                    nc.vector.tensor_reduce(
                        gm[:, 5:30], t1[:, :].rearrange("p (q i) -> p q i", i=3),
                        axis=AX.X, op=ALU.add)
                    t2 = tpool.tile([128, 150], BF16, tag="t2")
                    nc.vector.tensor_tensor(
                        t2[:, :].rearrange("p (r s q) -> p r s q", r=5, s=5),
                        m2u.unsqueeze(2).broadcast_to((128, 5, 5, 6)),
                        m2w.unsqueeze(1).broadcast_to((128, 5, 5, 6)), ALU.mult)
                    nc.vector.tensor_reduce(
                        gm[:, 30:55], t2[:, :].rearrange("p (q x) -> p q x", x=6),
                        axis=AX.X, op=ALU.add)
                    t3 = tpool.tile([128, 250], BF16, tag="t3")
                    nc.vector.tensor_tensor(
                        t3[:, :].rearrange("p (r s q) -> p r s q", r=5, s=5),
                        m3u.unsqueeze(2).broadcast_to((128, 5, 5, 10)),
                        m3w.unsqueeze(1).broadcast_to((128, 5, 5, 10)), ALU.mult)
                    nc.vector.tensor_reduce(
                        gm[:, 55:80], t3[:, :].rearrange("p (q x) -> p q x", x=10),
                        axis=AX.X, op=ALU.add)
                    tD = tpool.tile([128, 225], BF16, tag="tD")
                    nc.vector.tensor_tensor(
                        tD[:, :].rearrange("p (r j s i) -> p r j s i",
                                           r=5, j=3, s=5),
                        m2f.unsqueeze(3).broadcast_to((128, 5, 3, 5, 3)),
                        m1.unsqueeze(1).unsqueeze(1)
                            .broadcast_to((128, 5, 3, 5, 3)), ALU.mult)
                    D = dpool.tile([128, 75], BF16, tag="D")  # memory [r,s,j]
                    nc.vector.tensor_reduce(
                        D[:, :].rearrange("p (r s j) -> p r j s", r=5, s=5),
                        tD[:, :].rearrange("p (q i) -> p q i", i=3),
                        axis=AX.X, op=ALU.add)
                    t4 = tpool.tile([128, 375], BF16, tag="t4")
                    nc.vector.tensor_tensor(
                        t4[:, :].rearrange("p (r s t j) -> p r s t j",
                                               r=5, s=5, t=5),
                        D[:, :].rearrange("p (r s j) -> p r s j", r=5, s=5)
                            .unsqueeze(3).broadcast_to((128, 5, 5, 5, 3)),
                        m1.unsqueeze(1).unsqueeze(1)
                            .broadcast_to((128, 5, 5, 5, 3)), ALU.mult)
                    nc.vector.tensor_reduce(
                        gm[:, 80:205], t4[:, :].rearrange("p (q j) -> p q j", j=3),
                        axis=AX.X, op=ALU.add)
                    tF = tpool.tile([128, 675], BF16, tag="tF")
                    nc.gpsimd.tensor_tensor(
                        tF[:, :].rearrange("p (r k s q) -> p r k s q",
                                           r=5, k=3, s=5),
                        m3k.unsqueeze(3).broadcast_to((128, 5, 3, 5, 9)),
                        m2f9.unsqueeze(1).unsqueeze(1)
                            .broadcast_to((128, 5, 3, 5, 9)),
                        ALU.mult)
                    Ft = dpool.tile([128, 75], BF16, tag="Ft")  # memory [r,s,k]
                    nc.vector.tensor_reduce(
                        Ft[:, :].rearrange("p (r s k) -> p r k s", r=5, s=5),
                        tF[:, :].rearrange("p (q x) -> p q x", x=9),
                        axis=AX.X, op=ALU.add)
                    t5 = tpool.tile([128, 375], BF16, tag="t5")
                    nc.gpsimd.tensor_tensor(
                        t5[:, :].rearrange("p (r s t k) -> p r s t k",
                                               r=5, s=5, t=5),
                        Ft[:, :].rearrange("p (r s k) -> p r s k", r=5, s=5)
                            .unsqueeze(3).broadcast_to((128, 5, 5, 5, 3)),
                        m1.unsqueeze(1).unsqueeze(1)
                            .broadcast_to((128, 5, 5, 5, 3)), ALU.mult)
                    nc.vector.tensor_reduce(
                        gm[:, 205:330], t5[:, :].rearrange("p (q k) -> p q k", k=3),
                        axis=AX.X, op=ALU.add)

            # ---------------- readout MLP per group ----------------
            def emit_group(g):
                w = 4 * g
                nw = min(4, WPC - w)
                na = nw * 128
                gt = [gtpool.tile([128, 512], BF16, tag=f"gt{k}", name=f"gt{k}")
                      for k in range(3)]
                for wi in range(nw):
                    base = (w + wi) * 330
                    nc.sync.dma_start_transpose(
                        gt[0][0:128, wi * 128:(wi + 1) * 128],
                        gmAll[:, base:base + 128])
                    nc.sync.dma_start_transpose(
                        gt[1][0:128, wi * 128:(wi + 1) * 128],
                        gmAll[:, base + 128:base + 256])
                    nc.sync.dma_start_transpose(
                        gt[2][0:128, wi * 128:(wi + 1) * 128],
                        gmAll[:, base + 256:base + 384])
                hps = ph.tile([64, 512], F32, tag="hps")
                nc.tensor.matmul(hps[:, 0:na], wnB[0][:, :], gt[0][:, 0:na],
                                 start=True, stop=False)
                nc.tensor.matmul(hps[:, 0:na], wnB[1][:, :], gt[1][:, 0:na],
                                 start=False, stop=False)
                nc.tensor.matmul(hps[:, 0:na], wnB[2][:, :], gt[2][0:74, 0:na],
                                 start=False, stop=True)
                hT = hpool.tile([64, 512], F32R, tag="hT")
                nc.scalar.activation(hT[:, 0:na], hps[:, 0:na], ACTF.Identity,
                                     scale=S330)
                mps = ph.tile([64, 512], F32, tag="hps")
                nc.tensor.matmul(mps[:, 0:na], wmR[:, :], hT[:, 0:na],
                                 start=True, stop=True)
                msgT = hpool.tile([64, 512], F32R, tag="msgT")
                nc.scalar.activation(msgT[:, 0:na], mps[:, 0:na], ACTF.Silu,
                                     scale=S64)
                degt = hpool.tile([1, 512], F32, tag="degt")
                nc.sync.dma_start(degt[0:1, 0:na],
                                  dram["deg"][0:1, w * 128:w * 128 + na])
                degtR = hpool.tile([1, 512], F32R, tag="degtR")
                nc.scalar.copy(degtR[0:1, 0:na], degt[0:1, 0:na])
                dps = ph.tile([64, 512], F32, tag="hps")
                nc.tensor.matmul(dps[:, 0:na], onesR32[:, :],
                                 degtR[0:1, 0:na], start=True, stop=True)
                msgd = hpool.tile([64, 512], F32R, tag="msgd")
                nc.vector.tensor_tensor(msgd[:, 0:na], msgT[:, 0:na],
                                        dps[:, 0:na], ALU.mult)
                hpT = hpool.tile([64, 512], F32R, tag="hpT")
                nc.vector.tensor_tensor(hpT[:, 0:na], hT[:, 0:na],
                                        msgd[:, 0:na], ALU.add)
                a1 = [apool.tile([128, 512], F32R, tag=f"a1_{m}", name=f"a1_{m}")
                      for m in range(4)]
                for m in range(4):
                    ps = pmlp.tile([128, 512], F32)
                    nc.tensor.matmul(ps[:, 0:na], w1R[:, m * 128:(m + 1) * 128],
                                     hpT[:, 0:na], start=True, stop=True)
                    nc.scalar.activation(a1[m][:, 0:na], ps[:, 0:na], ACTF.Silu,
                                         bias=b1[:, m:m + 1], scale=S64)
                a2 = [a2pool.tile([128, 512], F32R, tag=f"a2_{m}", name=f"a2_{m}")
                      for m in range(4)]
                for m in range(4):
                    ps = pmlp.tile([128, 512], F32)
                    for k in range(4):
                        nc.tensor.matmul(ps[:, 0:na],
                                         w2R[k][:, m * 128:(m + 1) * 128],
                                         a1[k][:, 0:na],
                                         start=(k == 0), stop=(k == 3))
                    nc.scalar.activation(a2[m][:, 0:na], ps[:, 0:na], ACTF.Silu,
                                         bias=b2[:, m:m + 1], scale=S512)
                ops_ = ph.tile([64, 512], F32, tag="hps")
                for k in range(4):
                    nc.tensor.matmul(ops_[0:1, 0:na], w3R[k][:, :], a2[k][:, 0:na],
                                     start=(k == 0), stop=(k == 3))
                orow = hpool.tile([1, 512], F32, tag="orow")
                nc.scalar.activation(orow[0:1, 0:na], ops_[0:1, 0:na],
                                     ACTF.Identity, bias=b3[0:1, 0:1], scale=S512)
                nc.sync.dma_start(d_stage[0:1, w * 128:w * 128 + na],
                                  orow[0:1, 0:na])

            # ---------------- interleaved emission ----------------
            for b in range(NB):
                emit_geometry(b)
                for w in range(b * GB, (b + 1) * GB):
                    emit_window(w)
                    if (w + 1) % 4 == 0:
                        emit_group((w + 1) // 4 - 1)
            emit_group(12)

            # ---------------- epilogue ----------------
            outM = finpool.tile([128, WPC], F32)
            nc.sync.dma_start(outM[:, :],
                              d_stage.rearrange("q (w p) -> (q p) w", p=128))
            nc.vector.tensor_tensor(outM[:, :], outM[:, :], scsh[:, 0:WPC],
                                    ALU.mult)
            nc.vector.tensor_tensor(outM[:, :], outM[:, :], scsh[:, WPC:2 * WPC],
                                    ALU.add)
            nc.sync.dma_start(d_out[:, :], outM[:, :])

    nc.compile()
    return nc


_CACHED = {}


def kernel(dr_vec, Z, idx, W_node, W_msg, W_r1, b_r1, W_r2, b_r2, W_r3, b_r3,
           scale, shift):
    dr_vec = np.asarray(dr_vec, np.float32)
    Z = np.asarray(Z).astype(np.int64)
    i_idx = np.asarray(idx[0], np.int64)
    j_idx = np.asarray(idx[1], np.int64)

    C = int(np.ceil(np.bincount(i_idx >> 7, minlength=400).max() / 128))
    C3, C4 = 3 * C, 4 * C

    drP, irelP = _pack_edges(i_idx, dr_vec, C)

    deg = np.bincount(j_idx, minlength=N_PAD).astype(np.float32)

    mask = (Z != 0).astype(np.float32)
    scz = np.zeros(N_PAD, np.float32)
    shz = np.zeros(N_PAD, np.float32)
    scz[:N_ATOMS] = np.asarray(scale, np.float32).ravel()[Z] * mask
    shz[:N_ATOMS] = np.asarray(shift, np.float32).ravel()[Z] * mask

    common = dict(
        iota=np.broadcast_to(np.arange(128, dtype=np.float32)[None, :],
                             (128, 128)).copy(),
        ident=np.eye(128, dtype=np.float32),
        cent=np.broadcast_to(np.linspace(0.5, R_MAX, 5, dtype=np.float32)[None, :],
                             (128, 5)).copy(),
        ones=np.ones((1, 64), np.float32),
        emat=_build_E(),
        wn=np.asarray(W_node, np.float32), wm=np.asarray(W_msg, np.float32),
        w1=np.asarray(W_r1, np.float32), w2=np.asarray(W_r2, np.float32),
        w3=np.asarray(W_r3, np.float32),
        b1=np.asarray(b_r1, np.float32).reshape(512, 1),
        b2=np.asarray(b_r2, np.float32).reshape(512, 1),
        b3=np.asarray(b_r3, np.float32).reshape(1, 1),
    )
    in_maps = []
    for cc in range(N_CORES):
        w0 = cc * WPC
        m = dict(common)
        # edata: [128, WPC*C4] partition-major; per window [dr 3C | irel C]
        ed = np.empty((WPC, 128, C4), np.float32)
        ed[:, :, 0:C3] = drP[w0:w0 + WPC].reshape(WPC, 128, C3)
        ed[:, :, C3:C4] = irelP[w0:w0 + WPC]
        m["edata"] = np.ascontiguousarray(ed.transpose(1, 0, 2)).reshape(128, WPC * C4)
        m["deg"] = deg[cc * APC:(cc + 1) * APC].reshape(1, APC)
        sc = scz[cc * APC:(cc + 1) * APC].reshape(WPC, 128).T
        sh = shz[cc * APC:(cc + 1) * APC].reshape(WPC, 128).T
        m["scsh"] = np.ascontiguousarray(np.concatenate([sc, sh], axis=1))
        in_maps.append(m)

    key = (C,)
    if key not in _CACHED:
        _CACHED[key] = _build_program(C)
    nc = _CACHED[key]

    import time as _t
    t0 = _t.time()
    res = bass_utils.run_bass_kernel_spmd(
        nc, in_maps, core_ids=list(range(N_CORES)))
    t1 = _t.time()
    if os.environ.get("BENCH_TIME"):
        print(f"device run wall: {(t1 - t0) * 1e3:.1f} ms")
    if res.exec_time_ns is not None:
        print(f"HW exec time: {res.exec_time_ns} ns")
    outs = [res.results[c]["out"].T.ravel() for c in range(N_CORES)]
    full = np.concatenate(outs)[:N_ATOMS].astype(np.float32)
    return full[:, None]
